# revision 15
# baseline (speedup 1.0000x reference)
"""Trainium2 Bass kernel for causal multi-head attention.

Reference computation (B=2, T=2048, D=1024, H=16 heads, head_dim=64):
    q, k, v = x @ Wq, x @ Wk, x @ Wv         (per-head split)
    out = softmax(causal(q k^T / 8)) v  @ Wo

Sharding: 8 cores = 2 batches x 4 head-groups (4 heads each).  Each core
computes, for its batch b and its 4 heads:
    qT, kT [256, 2048] and v [2048, 256]  from the host-pre-transposed xT,
    transposed scores sT[tk, tq] = kT.T @ qT  (so softmax sums land on the
    matmul contraction axis and no on-chip transposes are ever needed),
    expS = exp(sT/8) * causal_mask,
    ctxT' [65, tq] = v'.T @ expS   with v' = [v | ones] so row 64 is the
    softmax denominator,
    ctxT_norm = ctxT * (1/rowsum)  (rank-1 PE broadcast of the reciprocal),
    partial_out [2048, 1024] = ctxT.T @ Wo[g*256:(g+1)*256, :].
Host sums the 4 partials per batch.

All matmuls run in bf16 (inputs host-converted): on real TRN2 hardware the
fp32r moving operand streams at 2 cycles/row while bf16 streams at 1.
The two heads of a pair live at SBUF partitions 0-63 / 64-127, so their
K=64 score matmuls run CONCURRENTLY in the PE (hardware row groups h0/h64)
and their scores land in one 2-bank PSUM tile, letting a single wide ACT
exp (and a single wide causal-mask multiply) cover the whole pair.  The
pair's ctx accumulators similarly share one 2-bank PSUM tile so the
end-of-pass drain is a single wide ACT copy.  ctx matmuls trail the score
matmuls by TWO iterations so the PE never waits on ACT exp latency.
Reciprocals run on ACT as exp(-ln(x)) (ln/exp share one table set),
keeping the DVE queue for mask/cast/drain work.

Scheduling: the next chunk's QKV projections, the previous chunk's
normalization and the output projections are interleaved into the
attention stream as fill work, distributed so no engine saturates.
"""

import sys

if "/opt/trn_rl_repo" not in sys.path:
    sys.path.insert(0, "/opt/trn_rl_repo")

import numpy as np

B, T, D, H = 2, 2048, 1024, 16
HD = 64                   # head dim
NCORES = 8
GROUPS = 4                # head groups (cores per batch)
HPC = H // GROUPS         # heads per core = 4
DHC = HPC * HD            # per-core head columns = 256
NKB = D // 128            # 8 contraction blocks for the projections
NTB = T // 128            # 16 t-blocks
NCH = T // 512            # 4 tq chunks of 512

_CACHE = {}


def _build():
    import concourse.bacc as bacc
    import concourse.tile as tile
    from concourse import mybir

    fp32 = mybir.dt.float32
    bf16 = mybir.dt.bfloat16
    Exp = mybir.ActivationFunctionType.Exp
    Ln = mybir.ActivationFunctionType.Ln

    nc = bacc.Bacc("TRN2", target_bir_lowering=False, debug=False,
                   num_devices=NCORES)

    xt_d = nc.dram_tensor("xt", [D, T], bf16, kind="ExternalInput")
    wq_d = nc.dram_tensor("wq", [D, DHC], bf16, kind="ExternalInput")
    wk_d = nc.dram_tensor("wk", [D, DHC], bf16, kind="ExternalInput")
    wv_d = nc.dram_tensor("wv", [D, DHC], bf16, kind="ExternalInput")
    wo_d = nc.dram_tensor("wo", [DHC, D], bf16, kind="ExternalInput")
    cm_d = nc.dram_tensor("cmask", [128, 1024], bf16, kind="ExternalInput")
    out_d = nc.dram_tensor("out", [T, D], fp32, kind="ExternalOutput")

    with tile.TileContext(nc) as tc:
        with (
            tc.tile_pool(name="consts", bufs=1) as consts,
            tc.tile_pool(name="xtp", bufs=2) as xtp,
            tc.tile_pool(name="big", bufs=1) as big,
            tc.tile_pool(name="es_pool", bufs=4) as es_pool,
            tc.tile_pool(name="small", bufs=3) as small,
            tc.tile_pool(name="outp", bufs=2) as outp,
            tc.tile_pool(name="psum", bufs=1, space="PSUM") as psum,
        ):
            wq_sb = consts.tile([128, NKB, DHC], bf16)
            wk_sb = consts.tile([128, NKB, DHC], bf16)
            wv_sb = consts.tile([128, NKB, DHC], bf16)
            wo_sb = consts.tile([128, 2, D], bf16)
            cm_sb = consts.tile([128, 1024], bf16)
            mk2 = consts.tile([128, 2, 256], bf16)
            dmy = consts.tile([128, 512], bf16)
            nc.gpsimd.memset(dmy, 0.0)
            wq_r = wq_d[:].rearrange("(k p) n -> p k n", p=128)
            nc.sync.dma_start(out=wq_sb[:, 0:4, :], in_=wq_r[:, 0:4, :])
            # HAM pre-warm: the PE clock-gate un-throttles only after ~3.4us
            # of sustained matmul activity.  The input DMAs take ~10us to
            # land, so burn that window on dummy matmuls -- the real QKV
            # stream then starts at the full 2.4 GHz clock.
            pdmy = psum.tile([128, 512], fp32, tag="mm", bufs=2,
                             name="pdmy")
            for _ in range(18):
                nc.tensor.matmul(pdmy, dmy[:, 0:128], dmy,
                                 start=True, stop=True)

            qt_sb = big.tile([128, 2, T], bf16)
            kt_sb = big.tile([128, 2, T], bf16)
            ct_sb = big.tile([128, 2, T], bf16)
            vs_sb = big.tile([128, NTB, HPC, HD + 1], bf16)

            xt_r = xt_d[:].rearrange("(k p) t -> p k t", p=128)
            xt_c = [None] * NCH
            # pcS[nj][hp]: pair ctxT' drained to SBUF at end of a pass
            pcS = [[None] * 2 for _ in range(NCH)]
            rsAll = [None] * NCH   # [97,512] per chunk: rowsums at rows 32h
            rcAll = [None] * NCH   # [97,512] bf16 per chunk: recips at rows 32h

            def load_xt(nj, eng=None):
                eng = eng or nc.sync
                xt_c[nj] = xtp.tile([128, NKB, 512], bf16, tag="xt",
                                    name=f"xt{nj}")
                half = NKB // 2
                eng.dma_start(out=xt_c[nj][:, 0:half, :],
                              in_=xt_r[:, 0:half, nj * 512:(nj + 1) * 512])
                eng.dma_start(out=xt_c[nj][:, half:, :],
                              in_=xt_r[:, half:, nj * 512:(nj + 1) * 512])

            def qkv_halves(nj):
                """16 closures, each half a psum accumulation group (4 MMs)."""
                cs = slice(nj * 512, (nj + 1) * 512)
                quanta = []

                def make_qk(wsb, dst, mb):
                    pq = [None]

                    def go_a():
                        pq[0] = psum.tile([128, 512], fp32, tag="mm", bufs=2,
                                          name=f"pq{nj}{mb}")
                        for kb in range(4):
                            nc.tensor.matmul(
                                pq[0],
                                wsb[:, kb, mb * 128:(mb + 1) * 128],
                                xt_c[nj][:, kb, :],
                                start=(kb == 0), stop=False,
                            )

                    def go_b():
                        for kb in range(4, NKB):
                            nc.tensor.matmul(
                                pq[0],
                                wsb[:, kb, mb * 128:(mb + 1) * 128],
                                xt_c[nj][:, kb, :],
                                start=False, stop=(kb == NKB - 1),
                            )
                        with nc.allow_low_precision(reason="bf16 qkv"):
                            nc.vector.tensor_copy(dst[:, mb, cs], pq[0])
                    return go_a, go_b

                def make_v(tb):
                    pv = [None]

                    def go_a():
                        pv[0] = psum.tile([128, 512], fp32, tag="mm", bufs=2,
                                          name=f"pv{tb}")
                        for kb in range(4):
                            nc.tensor.matmul(
                                pv[0][:, 0:DHC],
                                xt_c[nj][:, kb, (tb - 4 * nj) * 128:(tb - 4 * nj + 1) * 128],
                                wv_sb[:, kb, :],
                                start=(kb == 0), stop=False,
                            )

                    def go_b():
                        for kb in range(4, NKB):
                            nc.tensor.matmul(
                                pv[0][:, 0:DHC],
                                xt_c[nj][:, kb, (tb - 4 * nj) * 128:(tb - 4 * nj + 1) * 128],
                                wv_sb[:, kb, :],
                                start=False, stop=(kb == NKB - 1),
                            )
                        with nc.allow_low_precision(reason="bf16 qkv"):
                            nc.vector.tensor_copy(
                                vs_sb[:, tb, :, 0:HD],
                                pv[0][:, 0:DHC].rearrange("p (h d) -> p h d", h=HPC),
                            )
                    return go_a, go_b

                for mb in range(2):
                    quanta.extend(make_qk(wq_sb, qt_sb, mb))
                for mb in range(2):
                    quanta.extend(make_qk(wk_sb, kt_sb, mb))
                for tb in range(4 * nj, 4 * nj + 4):
                    quanta.extend(make_v(tb))
                return quanta

            rc3 = [None] * NCH

            def prenorm(nj, rows):
                """batched DVE reciprocal of chunk nj's rowsums (row range),
                split into four 128-column pieces so no single multi-us op
                blocks the strict-FIFO DVE queue (where it would stall the
                causal-mask and drain copies queued behind it).  Returns a
                list of fill closures.  h3's recip row (96) is relocated to
                partition 0 since matmul operands must start at 0/32/64."""
                r0, r1 = rows

                def piece(c):
                    def go():
                        with nc.allow_low_precision(reason="bf16 recip"):
                            nc.vector.reciprocal(
                                out=rcAll[nj][r0:r1, 128 * c:128 * (c + 1)],
                                in_=rsAll[nj][r0:r1, 128 * c:128 * (c + 1)])
                    return go

                def relocate():
                    rc3[nj] = small.tile([1, 512], bf16, tag="rc3",
                                         bufs=2, name=f"rc3_{nj}")
                    nc.vector.tensor_copy(rc3[nj], rcAll[nj][96:97, :])

                out = [piece(c) for c in range(4)]
                if r1 == 97:
                    out.append(relocate)
                return out

            def norm_fill(nj, h):
                """normalize head h of chunk nj from the SBUF-drained ctxT'."""
                def go():
                    mbh, ro = h >> 1, (h & 1) * 64
                    src = pcS[nj][h >> 1]
                    pb = psum.tile([64, 512], fp32, tag="mm", bufs=2,
                                   name=f"pb{nj}{h}")
                    if h == 3:
                        nc.tensor.matmul(pb, cm_sb[0:1, 512:576], rc3[nj],
                                         start=True, stop=True)
                    else:
                        nc.tensor.matmul(pb, cm_sb[32 * h:32 * h + 1, 512:576],
                                         rcAll[nj][32 * h:32 * h + 1, :],
                                         start=True, stop=True)
                    with nc.allow_low_precision(reason="bf16 ctx"):
                        nc.vector.tensor_mul(
                            ct_sb[ro:ro + 64, mbh, nj * 512:(nj + 1) * 512],
                            src[:, h & 1, :], pb)
                return go

            def outproj_fill(nj, tb, act_split=False):
                def go():
                    ot = outp.tile([128, D], fp32, tag="ot", name=f"ot{tb}")
                    for nk in range(2):
                        po = psum.tile([128, 512], fp32, tag="mm", bufs=2,
                                       name=f"po{tb}{nk}")
                        for mb in range(2):
                            nc.tensor.matmul(
                                po,
                                ct_sb[:, mb, tb * 128:(tb + 1) * 128],
                                wo_sb[:, mb, nk * 512:(nk + 1) * 512],
                                start=(mb == 0), stop=(mb == 1),
                            )
                        if act_split and nk == 0:
                            nc.scalar.copy(ot[:, 0:512], po)
                        else:
                            nc.vector.tensor_copy(
                                ot[:, nk * 512:(nk + 1) * 512], po)
                    nc.sync.dma_start(out=out_d[tb * 128:(tb + 1) * 128, :],
                                      in_=ot)
                return go

            def norm_fills(nj):
                return prenorm(nj, (0, 97)) + \
                    [norm_fill(nj, h) for h in range(HPC)]

            def outproj_fills(nj, act_split=False):
                return [outproj_fill(nj, tb, act_split)
                        for tb in range(4 * nj, 4 * nj + 4)]

            # prologue: the compute-critical DMA chain (wq, xt0, wk, wv) on
            # the sync queue; later-needed tensors (xt1, cm, wo) issue in
            # parallel on the ACT hwdge queue.
            load_xt(0)
            nc.sync.dma_start(out=wq_sb[:, 4:, :], in_=wq_r[:, 4:, :])
            nc.sync.dma_start(out=wk_sb, in_=wk_d[:].rearrange("(k p) n -> p k n", p=128))
            nc.sync.dma_start(out=wv_sb, in_=wv_d[:].rearrange("(k p) n -> p k n", p=128))
            load_xt(1, eng=nc.scalar)
            nc.scalar.dma_start(out=cm_sb, in_=cm_d[:])
            nc.scalar.dma_start(out=wo_sb, in_=wo_d[:].rearrange("(k p) n -> p k n", p=128))
            # ones column of v' (cmask cols 512.. are all 1.0, dtype bf16)
            nc.vector.tensor_copy(
                vs_sb[:, :, :, 64],
                cm_sb[:, 512:512 + NTB * HPC].rearrange("p (a b) -> p a b", a=NTB),
            )
            # duplicated diagonal-mask block (cm cols 256:512) for the
            # one-TT-per-pair causal masking
            for j in range(2):
                nc.vector.tensor_copy(mk2[:, j, :], cm_sb[:, 256:512])
            for q in qkv_halves(0):
                q()

            for nj in range(NCH):
                nb = 4 * nj + 4     # causal: tk-blocks 0 .. nb-1
                if nj + 2 < NCH:
                    load_xt(nj + 2)
                # rowsum/recip tiles for this chunk (rows 32h per head)
                rsAll[nj] = small.tile([97, 512], fp32, tag="rsall", bufs=2,
                                       name=f"rs{nj}")
                rcAll[nj] = small.tile([97, 512], bf16, tag="rcall", bufs=2,
                                       name=f"rc{nj}")
                nc.gpsimd.memset(rsAll[nj], 1.0)
                # deferred fill work for this chunk's attention span,
                # distributed so no single engine saturates in any chunk:
                #   chunk nj:   QKV(nj+1) + norm(nj-1)
                #   chunk 2:    + outproj(0)
                #   chunk 3:    + outproj(1,2) + last chunk's pair-0 norm
                fill = list(qkv_halves(nj + 1)) if nj + 1 < NCH else []
                if nj >= 1:
                    fill = fill + norm_fills(nj - 1)
                if nj == 2:
                    fill += outproj_fills(0)
                if nj == NCH - 1:
                    fill += outproj_fills(1) + outproj_fills(2)
                    fill += prenorm(nj, (0, 33)) + \
                        [norm_fill(nj, 0), norm_fill(nj, 1)]
                fi = 0

                total_iters = 2 * nb
                it = 0
                for hp in range(2):
                    heads = (2 * hp, 2 * hp + 1)
                    pc = psum.tile([65, 2, 512], fp32, tag="acc", bufs=1,
                                   name=f"pc{nj}{hp}")
                    pend = []
                    for i in range(nb):
                        m = i - 4 * nj
                        # causal window: diagonal blocks only need cols >= wm
                        wm = 0 if m < 0 else 128 * m
                        ps = psum.tile([128, 2, 512], fp32, tag="sc", bufs=2,
                                       name=f"ps{nj}{hp}{i}")
                        for k, h in enumerate(heads):
                            mbh, ro = h >> 1, (h & 1) * 64
                            nc.tensor.matmul(
                                ps[:, k, wm:512],
                                kt_sb[ro:ro + 64, mbh, i * 128:(i + 1) * 128],
                                qt_sb[ro:ro + 64, mbh, nj * 512 + wm:(nj + 1) * 512],
                                start=True, stop=True,
                            )
                        es = es_pool.tile([128, 2, 512], bf16, tag="es",
                                          name=f"es{nj}{hp}{i}")
                        with nc.allow_low_precision(reason="bf16 softmax"):
                            nc.scalar.activation(out=es[:, :, wm:512],
                                                 in_=ps[:, :, wm:512],
                                                 func=Exp, scale=0.125)
                        if m >= 0:
                            # the diagonal 128-col block needs masking; one
                            # TT covers both heads via the duplicated mask
                            nc.vector.tensor_mul(
                                es[:, :, wm:wm + 128],
                                es[:, :, wm:wm + 128],
                                mk2[:, :, 128:256],
                            )
                        pend.append((es, wm, i))
                        if len(pend) > 2:
                            # ctx trails by 2 iterations: exp+mask have two
                            # full PE slots to complete before the PE needs
                            # their output
                            pes, pwm, pi = pend.pop(0)
                            for k, h in enumerate(heads):
                                nc.tensor.matmul(
                                    pc[:, k, pwm:512],
                                    vs_sb[:, pi, h, :],
                                    pes[:, k, pwm:512],
                                    start=(pi == 0), stop=False,
                                )
                        it += 1
                        while fi < min(len(fill),
                                       int(len(fill) * it / total_iters + 0.999)):
                            fill[fi]()
                            fi += 1
                    for n_, (pes, pwm, pi) in enumerate(pend):
                        last = n_ == len(pend) - 1
                        for k, h in enumerate(heads):
                            nc.tensor.matmul(
                                pc[:, k, pwm:512],
                                vs_sb[:, pi, h, :],
                                pes[:, k, pwm:512],
                                start=(pi == 0), stop=last,
                            )
                    # drain this pass's ctxT' + rowsums to SBUF: two small
                    # DVE copies for the rowsum rows FIRST (so they run in
                    # parallel with the wide ACT ctx copy, not after it),
                    # then one wide ACT copy for both heads' ctx
                    for k, h in enumerate(heads):
                        nc.vector.tensor_copy(
                            rsAll[nj][32 * h:32 * h + 1, :],
                            pc[64:65, k, :])
                    dst = small.tile([64, 2, 512], bf16, tag="pcs", bufs=4,
                                     name=f"pcS{nj}{hp}")
                    with nc.allow_low_precision(reason="bf16 ctx"):
                        nc.scalar.copy(dst, pc[0:64, :, :])
                    pcS[nj][hp] = dst

                while fi < len(fill):
                    fill[fi]()
                    fi += 1

            # last chunk's tail: pair-1 norm + its output projections
            for go in (prenorm(NCH - 1, (64, 97))
                       + [norm_fill(NCH - 1, 2), norm_fill(NCH - 1, 3)]
                       + outproj_fills(NCH - 1, act_split=True)):
                go()

    nc.compile()
    return nc


def _causal_mask_block():
    # [128, 1024]: cols 0..383 = 0, cols 384..511 = upper-tri (p <= c-384),
    # cols 512.. = 1.
    m = np.zeros((128, 1024), np.float32)
    m[:, 512:] = 1.0
    m[:, 384:512] = np.triu(np.ones((128, 128), np.float32))
    return m


def _prepare_in_maps(x_q, Wq, Wk, Wv, Wo):
    import ml_dtypes
    bf16 = ml_dtypes.bfloat16

    x_q = np.asarray(x_q, np.float32)
    Wq = np.asarray(Wq, bf16)
    Wk = np.asarray(Wk, bf16)
    Wv = np.asarray(Wv, bf16)
    Wo = np.asarray(Wo, bf16)

    cmask = _causal_mask_block().astype(bf16)
    xts = [np.ascontiguousarray(x_q[b].T.astype(bf16)) for b in range(B)]
    in_maps = []
    for c in range(NCORES):
        b, g = divmod(c, GROUPS)
        sl = slice(g * DHC, (g + 1) * DHC)
        in_maps.append({
            "xt": xts[b],
            "wq": np.ascontiguousarray(Wq[:, sl]),
            "wk": np.ascontiguousarray(Wk[:, sl]),
            "wv": np.ascontiguousarray(Wv[:, sl]),
            "wo": np.ascontiguousarray(Wo[sl, :]),
            "cmask": cmask,
        })
    return in_maps


def _gather(results):
    out = np.zeros((B, T, D), np.float32)
    for c in range(NCORES):
        out[c // GROUPS] += np.asarray(results[c]["out"], np.float32)
    return out


def get_nc():
    if "nc" not in _CACHE:
        _CACHE["nc"] = _build()
    return _CACHE["nc"]


def kernel(x_q, Wq, Wk, Wv, Wo):
    from concourse.bass_utils import run_bass_kernel_spmd

    nc = get_nc()
    in_maps = _prepare_in_maps(x_q, Wq, Wk, Wv, Wo)
    res = run_bass_kernel_spmd(nc, in_maps, list(range(NCORES)))
    return _gather(res.results)
